# revision 20
# baseline (speedup 1.0000x reference)
"""Trainium2 Bass kernel for the ChunkAggregator problem.

Reference computation (B=8, L=8192, V=32000, D=128, BLOCK=64):
  blocks  = tokens.reshape(B, 128, 64)
  cat_ids = blocks[:, :, 0]                       # (B, 128)        int32
  cat_emb = cat_W[cat_ids]                        # (B, 128, D)
  hist    = scatter-add one-hot histogram         # (B, 128, V)     f32
  num_emb = hist @ num_W  == per-block sum of num_W[token]  # (B, 128, D)
  tok_emb = token_W[tokens]                       # (B, L, D)
  new_seq = concat([cat_emb, num_emb, tok_emb], axis=1)  # (B, 8448, D)
  returns (new_seq, cat_ids, hist)

Sharding: data-parallel over batch B across the 8 NeuronCores (one batch row
per core). Weight tables replicated. No collectives.

Device algorithm per core (batch row b):
  * two dma_gathers (8192 indices each, 512B rows) fetch token_W and num_W
    rows in block-major element order i = k*128 + n, so gathered layout is
    [p=block, k, 128] with fully contiguous per-partition data.
  * the token_W gather DMAs straight out to new_seq rows 256.. (one
    contiguous 32KB block per partition); the num_W gather is summed over k
    with a contiguous halving tree (6 tensor_adds) -> num_emb rows 128..255.
    (In-place tensor_adds on 3-D strided slices of an interleaved tile
    crashed the DVE exec unit on HW -- keep these ops 2-D contiguous.)
  * cat rows: 128-idx dma_gather of cat_W -> rows 0..127.
  * hist: viewed as (128*250, 128) rows; each token contributes a 128-wide
    one-hot of (token%128) scatter-added to row block*250 + token//128.
    dma_scatter_add does NOT accumulate correctly when two in-flight
    descriptors target the same row (CCE read-modify-write reads stale data),
    so duplicate (block, slot) keys are handled by construction:
      - occurrence 0 of each key -> "band 0" (cols 0..63 of the one-hot tile)
      - occurrence 1 -> band 1 at the SAME (partition, column) position;
        one tensor_add merges band1 into band0, so the merged vectors carry
        both counts and every band-0 scatter target is distinct.
      - occurrences 2..  -> small overflow scatter calls (distinct targets
        within each call), serialized after the main scatter by Tile's
        write-after-write ordering on hist.
    Pad cells carry rlow=-1 which never matches the iota compare -> zero
    vectors, scattered harmlessly to row 0.
  * one-hots built on device: iota row (j=0..127, int16) broadcast-compared
    (is_equal) against rlow = token%128 per cell.
  * hist zeros: the run paths pre-zero ExternalOutput buffers (documented
    contract of run_bass_kernel_spmd / bass2jax donation), so untouched rows
    are already zero; set ZERO_FILL=True to write them explicitly instead.

Host prepares only int16 DMA index/position arrays (wrapped [16, n/16] layout
tiled to 128 partitions) -- all f32 values are computed on device.
"""

import sys

import numpy as np

if "/opt/trn_rl_repo" not in sys.path:
    sys.path.insert(0, "/opt/trn_rl_repo")

B, L, V, D = 8, 8192, 32000, 128
BK = 64                # BLOCK
NB = L // BK           # 128 blocks per batch row
P = 128                # SBUF partitions
NS = 2 * NB + L        # 8448 rows of new_seq per batch row
SLOTS = V // D         # 250 128-float scatter rows per block
N_CORES = 8
ZCHUNK = 8000          # hist zero-fill free-dim chunk (when ZERO_FILL)
ZERO_FILL = False      # rely on pre-zeroed output buffers by default
# dma_gather/dma_scatter_add default single_packet=True, which violates the
# 64-descriptors-per-packet DMA limit above 1024 indices and crashes the
# device; all >1024-index calls must pass single_packet=False.
SCRATCH = 32768        # SWDGE descriptor carveout bytes/partition (sim ring)

_CACHE = {}


def _group_layout(extra_groups=0):
    """Scatter group sizes (elements). Group 0 = merged bands (occ 0+1);
    groups 1.. handle occurrence 2, 3, 4, ... of a (block, slot) key."""
    caps = [L, 512] + [128] * (3 + extra_groups)
    cols = [c // P for c in caps]          # one-hot tile columns per group
    return caps, cols


def build_nc(extra_groups=0):
    import concourse.bacc as bacc
    import concourse.mybir as mybir
    import concourse.tile as tile
    from concourse.alu_op_type import AluOpType

    caps, cols = _group_layout(extra_groups)
    n_cols = BK + sum(cols)            # band1 occupies BK extra columns
    sc_cols16 = sum(caps) // 16        # idx_sc columns (16 idx per column)

    nc = bacc.Bacc("TRN2", dynamic_dma_scratch_size=SCRATCH)

    tokens = nc.declare_dram_parameter("tokens", [NB, BK], mybir.dt.int32, isOutput=False)
    wcat = nc.declare_dram_parameter("wcat", [V, D], mybir.dt.float32, isOutput=False)
    wtok = nc.declare_dram_parameter("wtok", [V, D], mybir.dt.float32, isOutput=False)
    wnum = nc.declare_dram_parameter("wnum", [V, D], mybir.dt.float32, isOutput=False)
    idx_blk = nc.declare_dram_parameter("idx_blk", [P, L // 16], mybir.dt.int16, isOutput=False)
    idx_cat = nc.declare_dram_parameter("idx_cat", [P, NB // 16], mybir.dt.int16, isOutput=False)
    idx_sc = nc.declare_dram_parameter("idx_sc", [P, sc_cols16], mybir.dt.int16, isOutput=False)
    rlow = nc.declare_dram_parameter("rlow", [P, n_cols], mybir.dt.int16, isOutput=False)

    seq = nc.declare_dram_parameter("seq", [NS, D], mybir.dt.float32, isOutput=True)
    cids = nc.declare_dram_parameter("cids", [NB, 1], mybir.dt.int32, isOutput=True)
    hist = nc.declare_dram_parameter("hist", [NB, V], mybir.dt.float32, isOutput=True)

    with tile.TileContext(nc) as tc:
        with tc.tile_pool(name="pool", bufs=1) as pool:
            # -------- loads --------
            tok0 = pool.tile([P, 1], mybir.dt.int32)
            nc.sync.dma_start(tok0[:], tokens[:, 0:1])
            ib = pool.tile([P, L // 16], mybir.dt.int16)
            nc.sync.dma_start(ib[:], idx_blk[:])
            ic = pool.tile([P, NB // 16], mybir.dt.int16)
            nc.sync.dma_start(ic[:], idx_cat[:])
            isc = pool.tile([P, sc_cols16], mybir.dt.int16)
            nc.sync.dma_start(isc[:], idx_sc[:])
            rl = pool.tile([P, n_cols], mybir.dt.int16)
            nc.sync.dma_start(rl[:], rlow[:])

            # -------- cat_ids output --------
            nc.sync.dma_start(cids[:], tok0[:])

            # -------- one-hot(token % 128) per scatter cell --------
            iota_t = pool.tile([P, D], mybir.dt.int16)
            nc.gpsimd.iota(iota_t[:], pattern=[[1, D]], channel_multiplier=0)
            oh = pool.tile([P, n_cols, D], mybir.dt.float32)
            nc.vector.tensor_tensor(
                out=oh[:],
                in0=iota_t[:].rearrange("p (o d) -> p o d", o=1).to_broadcast([P, n_cols, D]),
                in1=rl[:].to_broadcast([P, n_cols, D]),
                op=AluOpType.is_equal,
            )
            # merge band 1 (occurrence-1 one-hots) into band 0 (2-D contiguous)
            ohf = oh[:].rearrange("p c d -> p (c d)")
            nc.vector.tensor_add(
                ohf[:, 0:BK * D], ohf[:, 0:BK * D], ohf[:, BK * D:2 * BK * D]
            )

            # -------- optional explicit hist zero-fill --------
            if ZERO_FILL:
                zt = pool.tile([P, ZCHUNK], mybir.dt.float32)
                nc.vector.memset(zt[:], 0.0)
                for q in range(V // ZCHUNK):
                    nc.sync.dma_start(hist[:, q * ZCHUNK:(q + 1) * ZCHUNK], zt[:])

            # -------- gathers (block-major order: element i=k*128+n) --------
            tokt = pool.tile([P, BK, D], mybir.dt.float32)
            nc.gpsimd.dma_gather(tokt[:], wtok[:], ib[:], L, L, D, single_packet=False)
            numt = pool.tile([P, BK, D], mybir.dt.float32)
            nc.gpsimd.dma_gather(numt[:], wnum[:], ib[:], L, L, D, single_packet=False)
            catt = pool.tile([P, 1, D], mybir.dt.float32)
            nc.gpsimd.dma_gather(catt[:], wcat[:], ic[:], NB, NB, D)

            # -------- new_seq token part (rows 256..8447) --------
            seq_tok = seq[2 * NB:NS, :].rearrange("(n k) d -> n k d", k=BK)
            nc.sync.dma_start(seq_tok, tokt[:])

            # -------- num_emb = per-block sum of gathered num rows --------
            numf = numt[:].rearrange("p k d -> p (k d)")
            h = BK // 2
            while h >= 1:
                nc.vector.tensor_add(
                    numf[:, 0:h * D], numf[:, 0:h * D], numf[:, h * D:2 * h * D]
                )
                h //= 2
            nc.sync.dma_start(seq[NB:2 * NB, :], numf[:, 0:D])

            # -------- cat_emb rows 0..127 --------
            nc.sync.dma_start(seq[0:NB, :], catt[:, 0, :])

            # -------- hist scatter-add chain --------
            # dma_scatter_add crashes above 4096 indices even with
            # single_packet=False (2 descriptors per element vs the gather's
            # 1), so the 8192-element group 0 is emitted as two 4096 calls.
            # All group-0 targets are distinct so the split is race-free.
            hist_rows = hist[:].rearrange("n (s d) -> (n s) d", d=D)
            col = 2 * BK          # one-hot column offset of the current group
            icol = 0              # idx_sc column offset (16 idx per column)
            for g, cap in enumerate(caps):
                nch = max(1, cap // 4096)
                sz = cap // nch
                scc, sci = sz // P, sz // 16
                for c in range(nch):
                    src_col = (0 if g == 0 else col) + c * scc
                    nc.gpsimd.dma_scatter_add(
                        hist_rows,
                        oh[:, src_col:src_col + scc, :],
                        isc[:, icol + c * sci:icol + (c + 1) * sci],
                        sz, sz, D,
                        single_packet=sz <= 1024,
                    )
                if g > 0:
                    col += cols[g]
                icol += cap // 16

    nc.compile()
    return nc


def _wrap16(vals, rows=None):
    """int16 wrapped layout for dma_gather/scatter indices: idx i at
    [i%16, i//16], replicated across the 8 16-partition groups."""
    n = vals.shape[0]
    w = vals.reshape(n // 16, 16).T.astype(np.int16)
    return np.tile(w, (8, 1))


def _scatter_layout(tb, extra_groups):
    """Compute band/overflow placement for one batch row.

    Returns (rlow [P, n_cols] int16, idx_sc [P, sc_cols16] int16) or None if
    some (block, slot) key occurs more than 2 + len(overflow groups) times.
    """
    caps, cols = _group_layout(extra_groups)
    n_cols = BK + sum(cols)

    t = tb.reshape(-1)                                   # flat, index = n*64+k
    n_arr = np.arange(L, dtype=np.int64) >> 6            # block of each token
    key = n_arr * SLOTS + (t >> 7)                       # global slot row
    low = (t & 127).astype(np.int16)

    order = np.argsort(key, kind="stable")
    sk = key[order]
    new_run = np.ones(L, bool)
    new_run[1:] = sk[1:] != sk[:-1]
    run_id = np.cumsum(new_run) - 1                      # dense key rank, sorted order
    run_start = np.flatnonzero(new_run)
    occ = np.arange(L) - run_start[run_id]               # occurrence # within key

    rlow = np.full((P, n_cols), -1, np.int16)
    sc = []

    # group 0: occurrence 0 -> band 0; element m = run_id (distinct keys in
    # sorted order). occurrence 1 -> band 1 at the same (p, c).
    m0 = run_id[occ == 0]
    tok0 = order[occ == 0]
    rlow[m0 % P, m0 // P] = low[tok0]
    m1 = run_id[occ == 1]
    tok1 = order[occ == 1]
    rlow[m1 % P, BK + m1 // P] = low[tok1]
    slots0 = np.zeros(caps[0], np.int64)
    slots0[m0] = sk[occ == 0]
    sc.append(slots0)

    col = 2 * BK
    for g in range(1, len(caps)):
        sel = occ == g + 1
        cnt = int(sel.sum())
        if cnt > caps[g]:
            return None
        mg = np.arange(cnt)
        tokg = order[sel]
        rlow[mg % P, col + mg // P] = low[tokg]
        slots_g = np.zeros(caps[g], np.int64)
        slots_g[mg] = sk[sel]
        sc.append(slots_g)
        col += cols[g]
    if (occ > len(caps)).any():
        return None

    idx_sc = np.concatenate([_wrap16(s) for s in sc], axis=1)
    return rlow, idx_sc


def _make_in_maps(tokens, cat_W, num_W, token_W, extra_groups):
    t32 = np.ascontiguousarray(np.asarray(tokens, dtype=np.int32))
    wcat = np.ascontiguousarray(np.asarray(cat_W, dtype=np.float32))
    wtok = np.ascontiguousarray(np.asarray(token_W, dtype=np.float32))
    wnum = np.ascontiguousarray(np.asarray(num_W, dtype=np.float32))
    in_maps = []
    for b in range(N_CORES):
        tb = t32[b].reshape(NB, BK)
        lay = _scatter_layout(tb, extra_groups)
        if lay is None:
            return None
        rlow, idx_sc = lay
        # gather element order i = k*128 + n (partition = block)
        in_maps.append({
            "tokens": tb,
            "wcat": wcat,
            "wtok": wtok,
            "wnum": wnum,
            "idx_blk": _wrap16(tb.T.reshape(-1)),
            "idx_cat": _wrap16(tb[:, 0]),
            "idx_sc": idx_sc,
            "rlow": rlow,
        })
    return in_maps


def kernel(tokens, cat_W, num_W, token_W):
    from concourse.bass_utils import run_bass_kernel_spmd

    tokens = np.asarray(tokens)
    tok_dtype = tokens.dtype

    extra = _CACHE.get("extra_groups", 0)
    while True:
        in_maps = _make_in_maps(tokens, cat_W, num_W, token_W, extra)
        if in_maps is not None:
            break
        extra += 2          # pathological duplicate depth: widen overflow chain
    if _CACHE.get("extra_groups") != extra or "nc" not in _CACHE:
        _CACHE["nc"] = build_nc(extra)
        _CACHE["extra_groups"] = extra
    nc = _CACHE["nc"]

    out = run_bass_kernel_spmd(nc, in_maps, core_ids=list(range(N_CORES)))
    _CACHE["last_result"] = out
    results = out.results

    new_seq = np.stack([r["seq"] for r in results])                      # (8, 8448, 128)
    cat_ids = np.stack([r["cids"].reshape(NB) for r in results])         # (8, 128)
    hist = np.stack([r["hist"] for r in results])                        # (8, 128, 32000)
    return new_seq.astype(np.float32), cat_ids.astype(tok_dtype), hist.astype(np.float32)


# revision 24
# speedup vs baseline: 1.4135x; 1.4135x over previous
"""Trainium2 Bass kernel for the ChunkAggregator problem.

Reference computation (B=8, L=8192, V=32000, D=128, BLOCK=64):
  blocks  = tokens.reshape(B, 128, 64)
  cat_ids = blocks[:, :, 0]                       # (B, 128)        int32
  cat_emb = cat_W[cat_ids]                        # (B, 128, D)
  hist    = scatter-add one-hot histogram         # (B, 128, V)     f32
  num_emb = hist @ num_W  == per-block sum of num_W[token]  # (B, 128, D)
  tok_emb = token_W[tokens]                       # (B, L, D)
  new_seq = concat([cat_emb, num_emb, tok_emb], axis=1)  # (B, 8448, D)
  returns (new_seq, cat_ids, hist)

Sharding: data-parallel over batch B across the 8 NeuronCores (one batch row
per core). Weight tables replicated. No collectives.

Device algorithm per core (batch row b). The kernel is SWDGE-gen bound
(the Q7 descriptor-generation loop costs ~9 ns/index), so the design
minimizes gather/scatter descriptor counts and chains:

  * token_W and num_W are interleaved host-side into one (V, 256) table so a
    single 8192-index dma_gather (1KB rows, block-major element order
    i = k*128 + n -> [p=block, k, 0:128]=token_W row, [.., 128:256]=num_W row)
    fetches both. The token half DMAs straight out to new_seq rows 256..;
    the num half is reduced over k (strided first add into a contiguous
    tile, then a contiguous halving tree) -> num_emb rows 128..255.
  * cat rows: 128-idx dma_gather of cat_W -> rows 0..127.
  * hist is emitted as TWO output tensors (blocks 0..63 -> hist_a, 64..127
    -> hist_b, concatenated on host) so the two 4096-element scatter-adds
    and their overflow calls form two independent completion chains.
  * Within a half, hist rows are (block-local)*250 + token//128 of the half
    viewed as (16000, 128); each scattered element is a 128-wide one-hot of
    token%128.  dma_scatter_add does NOT accumulate when two in-flight
    descriptors hit the same row (CCE RMW reads stale data), so duplicate
    (block, slot) keys are merged by construction:
      - occurrence 0 -> band 0 (one-hot tile cols 0..63; a: 0..31, b: 32..63)
      - occurrence 1 -> band 1 (cols 64..127) at the same (partition, col
        offset); one tensor_add merges band1 into band0.
      - occurrences 2..5 -> overflow bands (cols 128..143, 4 positional
        levels x 2 halves x 2 cols); 3 adds merge levels 1..3 into level 0,
        then ONE chained 256-element scatter per half applies them.
      - deeper keys (not present in practice) -> extra chained groups via
        a host-triggered rebuild with extra_groups > 0.
    Pad cells carry rlow=-1 which never matches the iota compare -> zero
    vectors, scatter-added harmlessly to row 0 of the half.
  * one-hots built on device: int16 iota row broadcast-compared (is_equal)
    against rlow = token%128 per cell.
  * hist zeros: the run paths pre-zero ExternalOutput buffers (documented
    contract of run_bass_kernel_spmd / bass2jax donation), so untouched rows
    are already zero; ZERO_FILL=True writes them explicitly instead.
  * dma_gather/dma_scatter_add must use single_packet=False above 1024
    indices (64-descriptor packet limit; scatter caps out at 4096 indices
    per call even then).

Host prepares only int16 DMA index/position arrays (wrapped [16, n/16]
layout tiled to 128 partitions) -- all f32 values are computed on device.
"""

import sys

import numpy as np

if "/opt/trn_rl_repo" not in sys.path:
    sys.path.insert(0, "/opt/trn_rl_repo")

B, L, V, D = 8, 8192, 32000, 128
BK = 64                # BLOCK
NB = L // BK           # 128 blocks per batch row
P = 128                # SBUF partitions
NS = 2 * NB + L        # 8448 rows of new_seq per batch row
SLOTS = V // D         # 250 128-float scatter rows per block
HB = NB // 2           # blocks per hist half
HROWS = HB * SLOTS     # 16000 scatter rows per half
N_CORES = 8
ZCHUNK = 8000
ZERO_FILL = False      # rely on pre-zeroed output buffers by default
SCRATCH = 24576        # SWDGE descriptor carveout bytes/partition
OVCAP = 256            # per-half overflow elements (occ 2..5, positional)
NLEV = 4               # overflow occurrence levels handled by banding (2..5)

_CACHE = {}


def _cols(extra_groups=0):
    """One-hot tile column layout. Returns (n_cols, extra_col0)."""
    # 64 band0 + 64 band1 + NLEV levels * (2 halves * OVCAP/P cols)
    ov_cols = NLEV * 2 * (OVCAP // P)
    base = 2 * BK + ov_cols
    return base + 2 * extra_groups, base


def _sc_caps(extra_groups=0):
    """Scatter calls in emission order: (name, cap, half). Group-0 halves
    first (independent chains), then per-half overflow, then extras."""
    calls = [("s0a", L // 2, 0), ("s0b", L // 2, 1), ("ova", OVCAP, 0), ("ovb", OVCAP, 1)]
    for e in range(extra_groups):
        calls.append((f"xa{e}", P, 0))
        calls.append((f"xb{e}", P, 1))
    return calls


def build_nc(extra_groups=0):
    import concourse.bacc as bacc
    import concourse.mybir as mybir
    import concourse.tile as tile
    from concourse.alu_op_type import AluOpType

    n_cols, extra_col0 = _cols(extra_groups)
    calls = _sc_caps(extra_groups)
    sc_cols16 = sum(c[1] for c in calls) // 16
    ovc = OVCAP // P          # cols per half per overflow level (2)

    nc = bacc.Bacc("TRN2", dynamic_dma_scratch_size=SCRATCH)

    tokens = nc.declare_dram_parameter("tokens", [NB, BK], mybir.dt.int32, isOutput=False)
    wcat = nc.declare_dram_parameter("wcat", [V, D], mybir.dt.float32, isOutput=False)
    wcomb = nc.declare_dram_parameter("wcomb", [V, 2 * D], mybir.dt.float32, isOutput=False)
    idx_blk = nc.declare_dram_parameter("idx_blk", [P, L // 16], mybir.dt.int16, isOutput=False)
    idx_cat = nc.declare_dram_parameter("idx_cat", [P, NB // 16], mybir.dt.int16, isOutput=False)
    idx_sc = nc.declare_dram_parameter("idx_sc", [P, sc_cols16], mybir.dt.int16, isOutput=False)
    rlow = nc.declare_dram_parameter("rlow", [P, n_cols], mybir.dt.int16, isOutput=False)

    seq = nc.declare_dram_parameter("seq", [NS, D], mybir.dt.float32, isOutput=True)
    cids = nc.declare_dram_parameter("cids", [NB, 1], mybir.dt.int32, isOutput=True)
    hist_a = nc.declare_dram_parameter("hist_a", [HB, V], mybir.dt.float32, isOutput=True)
    hist_b = nc.declare_dram_parameter("hist_b", [HB, V], mybir.dt.float32, isOutput=True)
    halves = [hist_a, hist_b]

    with tile.TileContext(nc) as tc:
        with tc.tile_pool(name="pool", bufs=1) as pool:
            # -------- loads --------
            tok0 = pool.tile([P, 1], mybir.dt.int32)
            nc.sync.dma_start(tok0[:], tokens[:, 0:1])
            ib = pool.tile([P, L // 16], mybir.dt.int16)
            nc.sync.dma_start(ib[:], idx_blk[:])
            ic = pool.tile([P, NB // 16], mybir.dt.int16)
            nc.sync.dma_start(ic[:], idx_cat[:])
            isc = pool.tile([P, sc_cols16], mybir.dt.int16)
            nc.sync.dma_start(isc[:], idx_sc[:])
            rl = pool.tile([P, n_cols], mybir.dt.int16)
            nc.sync.dma_start(rl[:], rlow[:])

            # -------- cat_ids output --------
            nc.sync.dma_start(cids[:], tok0[:])

            # -------- one-hot(token % 128) per scatter cell --------
            iota_t = pool.tile([P, D], mybir.dt.int16)
            nc.gpsimd.iota(iota_t[:], pattern=[[1, D]], channel_multiplier=0)
            oh = pool.tile([P, n_cols, D], mybir.dt.float32)
            nc.vector.tensor_tensor(
                out=oh[:],
                in0=iota_t[:].rearrange("p (o d) -> p o d", o=1).to_broadcast([P, n_cols, D]),
                in1=rl[:].to_broadcast([P, n_cols, D]),
                op=AluOpType.is_equal,
            )
            ohf = oh[:].rearrange("p c d -> p (c d)")
            # band 1 -> band 0 (both halves at once)
            nc.vector.tensor_add(
                ohf[:, 0:BK * D], ohf[:, 0:BK * D], ohf[:, BK * D:2 * BK * D]
            )
            # overflow levels 1..3 -> level 0
            lv0 = 2 * BK * D
            lw = 2 * ovc * D          # elements per overflow level (both halves)
            for j in range(1, NLEV):
                nc.vector.tensor_add(
                    ohf[:, lv0:lv0 + lw],
                    ohf[:, lv0:lv0 + lw],
                    ohf[:, lv0 + j * lw:lv0 + (j + 1) * lw],
                )

            # -------- optional explicit hist zero-fill --------
            if ZERO_FILL:
                zt = pool.tile([P, ZCHUNK], mybir.dt.float32)
                nc.vector.memset(zt[:], 0.0)
                for h_ in halves:
                    for q in range(V // ZCHUNK):
                        nc.sync.dma_start(
                            h_[:, q * ZCHUNK:(q + 1) * ZCHUNK], zt[0:HB, :]
                        )

            # -------- combined gather (element order i = k*128 + n) --------
            comb = pool.tile([P, BK, 2 * D], mybir.dt.float32)
            nc.gpsimd.dma_gather(comb[:], wcomb[:], ib[:], L, L, 2 * D, single_packet=False)
            catt = pool.tile([P, 1, D], mybir.dt.float32)
            nc.gpsimd.dma_gather(catt[:], wcat[:], ic[:], NB, NB, D)

            # -------- new_seq token part (rows 256..8447) --------
            seq_tok = seq[2 * NB:NS, :].rearrange("(n k) d -> n k d", k=BK)
            nc.sync.dma_start(seq_tok, comb[:, :, 0:D])

            # -------- num_emb = per-block sum of num rows --------
            numc = pool.tile([P, (BK // 2) * D], mybir.dt.float32)
            nv = numc[:].rearrange("p (k d) -> p k d", d=D)
            nc.vector.tensor_add(
                nv[:, 0:BK // 2, :], comb[:, 0:BK // 2, D:2 * D],
                comb[:, BK // 2:BK, D:2 * D],
            )
            h = BK // 4
            while h >= 1:
                nc.vector.tensor_add(
                    numc[:, 0:h * D], numc[:, 0:h * D], numc[:, h * D:2 * h * D]
                )
                h //= 2
            nc.sync.dma_start(seq[NB:2 * NB, :], numc[:, 0:D])

            # -------- cat_emb rows 0..127 --------
            nc.sync.dma_start(seq[0:NB, :], catt[:, 0, :])

            # -------- hist scatter-add (two independent half-chains) --------
            rows = [h_[:].rearrange("n (s d) -> (n s) d", d=D) for h_ in halves]
            icol = 0
            for name, cap, half in calls:
                if name.startswith("s0"):
                    c0 = half * (HB // 2)                     # 0 or 32
                elif name.startswith("ov"):
                    c0 = 2 * BK + half * ovc                  # level-0 half slice
                else:
                    e = int(name[2:])
                    c0 = extra_col0 + 2 * e + half
                nc.gpsimd.dma_scatter_add(
                    rows[half], oh[:, c0:c0 + cap // P, :],
                    isc[:, icol:icol + cap // 16], cap, cap, D,
                    single_packet=cap <= 1024,
                )
                icol += cap // 16

    nc.compile()
    return nc


def _wrap16(vals):
    """int16 wrapped index layout: idx i at [i%16, i//16], tiled to 128 rows."""
    n = vals.shape[0]
    w = vals.reshape(n // 16, 16).T.astype(np.int16)
    return np.tile(w, (8, 1))


def _scatter_layout(tb, extra_groups):
    """Band/overflow placement for one batch row. Returns (rlow, idx_sc) or
    None if some key is deeper than the configured groups."""
    n_cols, extra_col0 = _cols(extra_groups)
    calls = _sc_caps(extra_groups)
    ovc = OVCAP // P

    rlow = np.full((P, n_cols), -1, np.int16)
    sc = {}

    for half in (0, 1):
        hb0 = half * HB
        th = tb[hb0:hb0 + HB].reshape(-1)                    # 4096 tokens
        n_loc = np.arange(HB * BK, dtype=np.int64) >> 6      # local block
        key = n_loc * SLOTS + (th >> 7)                      # half-local row
        low = (th & 127).astype(np.int16)

        order = np.argsort(key, kind="stable")
        sk = key[order]
        new_run = np.ones(HB * BK, bool)
        new_run[1:] = sk[1:] != sk[:-1]
        run_id = np.cumsum(new_run) - 1
        run_start = np.flatnonzero(new_run)
        occ = np.arange(HB * BK) - run_start[run_id]
        if (occ >= 2 + NLEV + extra_groups).any():
            return None

        c_b0 = half * (HB // 2)                              # 0 / 32
        # occurrence 0 -> band 0; 1 -> band 1
        m0 = run_id[occ == 0]
        rlow[m0 % P, c_b0 + m0 // P] = low[order[occ == 0]]
        m1 = run_id[occ == 1]
        rlow[m1 % P, BK + c_b0 + m1 // P] = low[order[occ == 1]]
        slots0 = np.zeros(L // 2, np.int64)
        slots0[m0] = sk[occ == 0]
        sc["s0a" if half == 0 else "s0b"] = slots0

        # overflow: occ 2..5 -> positional levels; m2 assigned at occ 2
        sel2 = np.flatnonzero(occ == 2)
        if sel2.size > OVCAP:
            return None
        m2_of = np.full(HB * BK, -1, np.int64)               # by sorted pos
        m2_of[sel2] = np.arange(sel2.size)
        ov_slots = np.zeros(OVCAP, np.int64)
        ov_slots[np.arange(sel2.size)] = sk[sel2]
        c_ov0 = 2 * BK + half * ovc
        for j in range(NLEV):
            q = np.flatnonzero(occ == 2 + j)
            if q.size == 0:
                continue
            m2 = m2_of[q - j]                                # occ-2 sibling
            rlow[m2 % P, c_ov0 + j * 2 * ovc + m2 // P] = low[order[q]]
        sc["ova" if half == 0 else "ovb"] = ov_slots

        # extras: occ >= 2+NLEV, one chained cap-128 group per depth level
        for e in range(extra_groups):
            q = np.flatnonzero(occ == 2 + NLEV + e)
            if q.size > P:
                return None
            xs = np.zeros(P, np.int64)
            xs[np.arange(q.size)] = sk[q]
            mg = np.arange(q.size)
            rlow[mg % P, extra_col0 + 2 * e + half] = low[order[q]]
            sc[f"xa{e}" if half == 0 else f"xb{e}"] = xs

    for name, cap, _ in calls:
        if name not in sc:
            sc[name] = np.zeros(cap, np.int64)
    idx_sc = np.concatenate([_wrap16(sc[name]) for name, _, _ in calls], axis=1)
    return rlow, idx_sc


def _make_in_maps(tokens, cat_W, num_W, token_W, extra_groups):
    t32 = np.ascontiguousarray(np.asarray(tokens, dtype=np.int32))
    wcat = np.ascontiguousarray(np.asarray(cat_W, dtype=np.float32))
    wcomb = np.ascontiguousarray(
        np.concatenate(
            [np.asarray(token_W, dtype=np.float32), np.asarray(num_W, dtype=np.float32)],
            axis=1,
        )
    )
    in_maps = []
    for b in range(N_CORES):
        tb = t32[b].reshape(NB, BK)
        lay = _scatter_layout(tb, extra_groups)
        if lay is None:
            return None
        rlow, idx_sc = lay
        in_maps.append({
            "tokens": tb,
            "wcat": wcat,
            "wcomb": wcomb,
            "idx_blk": _wrap16(tb.T.reshape(-1)),
            "idx_cat": _wrap16(tb[:, 0]),
            "idx_sc": idx_sc,
            "rlow": rlow,
        })
    return in_maps


def kernel(tokens, cat_W, num_W, token_W):
    from concourse.bass_utils import run_bass_kernel_spmd

    tokens = np.asarray(tokens)
    tok_dtype = tokens.dtype

    extra = _CACHE.get("extra_groups", 0)
    while True:
        in_maps = _make_in_maps(tokens, cat_W, num_W, token_W, extra)
        if in_maps is not None:
            break
        extra += 2          # pathological duplicate depth: widen chains
    if _CACHE.get("extra_groups") != extra or "nc" not in _CACHE:
        _CACHE["nc"] = build_nc(extra)
        _CACHE["extra_groups"] = extra
    nc = _CACHE["nc"]

    out = run_bass_kernel_spmd(nc, in_maps, core_ids=list(range(N_CORES)))
    _CACHE["last_result"] = out
    results = out.results

    new_seq = np.stack([r["seq"] for r in results])
    cat_ids = np.stack([r["cids"].reshape(NB) for r in results])
    hist = np.stack(
        [np.concatenate([r["hist_a"], r["hist_b"]], axis=0) for r in results]
    )
    return new_seq.astype(np.float32), cat_ids.astype(tok_dtype), hist.astype(np.float32)


# revision 27
# speedup vs baseline: 1.5181x; 1.0741x over previous
"""Trainium2 Bass kernel for the ChunkAggregator problem.

Reference computation (B=8, L=8192, V=32000, D=128, BLOCK=64):
  blocks  = tokens.reshape(B, 128, 64)
  cat_ids = blocks[:, :, 0]                       # (B, 128)        int32
  cat_emb = cat_W[cat_ids]                        # (B, 128, D)
  hist    = scatter-add one-hot histogram         # (B, 128, V)     f32
  num_emb = hist @ num_W  == per-block sum of num_W[token]  # (B, 128, D)
  tok_emb = token_W[tokens]                       # (B, L, D)
  new_seq = concat([cat_emb, num_emb, tok_emb], axis=1)  # (B, 8448, D)
  returns (new_seq, cat_ids, hist)

Sharding: data-parallel over batch B across the 8 NeuronCores (one batch row
per core). Weight tables replicated. No collectives.

Device algorithm per core (batch row b). The kernel is SWDGE-gen bound
(the Q7 descriptor-generation loop costs ~9 ns/index), so the design
minimizes gather/scatter descriptor counts and chains:

  * token_W and num_W are interleaved host-side into one (V, 256) table so a
    single 8192-index dma_gather (1KB rows, block-major element order
    i = k*128 + n -> [p=block, k, 0:128]=token_W row, [.., 128:256]=num_W row)
    fetches both. The token half DMAs straight out to new_seq rows 256..;
    the num half is reduced over k (strided first add into a contiguous
    tile, then a contiguous halving tree) -> num_emb rows 128..255.
  * cat rows: 128-idx dma_gather of cat_W -> rows 0..127.
  * hist is emitted as TWO output tensors (blocks 0..63 -> hist_a, 64..127
    -> hist_b, concatenated on host) so the two 4096-element scatter-adds
    and their overflow calls form two independent completion chains.
  * Within a half, hist rows are (block-local)*250 + token//128 of the half
    viewed as (16000, 128); each scattered element is a 128-wide one-hot of
    token%128.  dma_scatter_add does NOT accumulate when two in-flight
    descriptors hit the same row (CCE RMW reads stale data), so duplicate
    (block, slot) keys are merged by construction:
      - occurrence 0 -> band 0 (one-hot tile cols 0..63; a: 0..31, b: 32..63)
      - occurrence 1 -> band 1 (cols 64..127) at the same (partition, col
        offset); one tensor_add merges band1 into band0.
      - occurrences 2..5 -> overflow bands (cols 128..143, 4 positional
        levels x 2 halves x 2 cols); 3 adds merge levels 1..3 into level 0,
        then ONE chained 256-element scatter per half applies them.
      - deeper keys (not present in practice) -> extra chained groups via
        a host-triggered rebuild with extra_groups > 0.
    Pad cells carry rlow=-1 which never matches the iota compare -> zero
    vectors, scatter-added harmlessly to row 0 of the half.
  * one-hots built on device: int16 iota row broadcast-compared (is_equal)
    against rlow = token%128 per cell.
  * hist zeros: the run paths pre-zero ExternalOutput buffers (documented
    contract of run_bass_kernel_spmd / bass2jax donation), so untouched rows
    are already zero; ZERO_FILL=True writes them explicitly instead.
  * dma_gather/dma_scatter_add must use single_packet=False above 1024
    indices (64-descriptor packet limit; scatter caps out at 4096 indices
    per call even then).

Host prepares only int16 DMA index/position arrays (wrapped [16, n/16]
layout tiled to 128 partitions) -- all f32 values are computed on device.
"""

import sys

import numpy as np

if "/opt/trn_rl_repo" not in sys.path:
    sys.path.insert(0, "/opt/trn_rl_repo")

B, L, V, D = 8, 8192, 32000, 128
BK = 64                # BLOCK
NB = L // BK           # 128 blocks per batch row
P = 128                # SBUF partitions
NS = 2 * NB + L        # 8448 rows of new_seq per batch row
SLOTS = V // D         # 250 128-float scatter rows per block
HB = NB // 2           # blocks per hist half
HROWS = HB * SLOTS     # 16000 scatter rows per half
N_CORES = 8
ZCHUNK = 8000
ZERO_FILL = False      # rely on pre-zeroed output buffers by default
SCRATCH = 32768        # SWDGE descriptor carveout bytes/partition
OVCAP = 256            # per-half overflow elements (occ 2..5, positional)
NLEV = 4               # overflow occurrence levels handled by banding (2..5)

_CACHE = {}


def _cols(extra_groups=0):
    """One-hot tile column layout. Returns (n_cols, extra_col0)."""
    # 64 band0 + 64 band1 + NLEV levels * (2 halves * OVCAP/P cols)
    ov_cols = NLEV * 2 * (OVCAP // P)
    base = 2 * BK + ov_cols
    return base + 2 * extra_groups, base


def _sc_caps(extra_groups=0):
    """Scatter calls in emission order: (name, cap, half). Group-0 halves
    first (independent chains), then per-half overflow, then extras."""
    calls = [("s0a", L // 2, 0), ("s0b", L // 2, 1), ("ova", OVCAP, 0), ("ovb", OVCAP, 1)]
    for e in range(extra_groups):
        calls.append((f"xa{e}", P, 0))
        calls.append((f"xb{e}", P, 1))
    return calls


def build_nc(extra_groups=0):
    import concourse.bacc as bacc
    import concourse.mybir as mybir
    import concourse.tile as tile
    from concourse.alu_op_type import AluOpType

    n_cols, extra_col0 = _cols(extra_groups)
    calls = _sc_caps(extra_groups)
    sc_cols16 = sum(c[1] for c in calls) // 16
    ovc = OVCAP // P          # cols per half per overflow level (2)

    nc = bacc.Bacc("TRN2", dynamic_dma_scratch_size=SCRATCH)

    tokens = nc.declare_dram_parameter("tokens", [NB, BK], mybir.dt.int32, isOutput=False)
    wcat = nc.declare_dram_parameter("wcat", [V, D], mybir.dt.float32, isOutput=False)
    wcomb = nc.declare_dram_parameter("wcomb", [V, 2 * D], mybir.dt.float32, isOutput=False)
    idx_blk = nc.declare_dram_parameter("idx_blk", [P, L // 16], mybir.dt.int16, isOutput=False)
    idx_cat = nc.declare_dram_parameter("idx_cat", [P, NB // 16], mybir.dt.int16, isOutput=False)
    idx_sc = nc.declare_dram_parameter("idx_sc", [P, sc_cols16], mybir.dt.int16, isOutput=False)
    rlow = nc.declare_dram_parameter("rlow", [P, n_cols], mybir.dt.int16, isOutput=False)

    seq = nc.declare_dram_parameter("seq", [NS, D], mybir.dt.float32, isOutput=True)
    cids = nc.declare_dram_parameter("cids", [NB, 1], mybir.dt.int32, isOutput=True)
    hist_a = nc.declare_dram_parameter("hist_a", [HB, V], mybir.dt.float32, isOutput=True)
    hist_b = nc.declare_dram_parameter("hist_b", [HB, V], mybir.dt.float32, isOutput=True)
    halves = [hist_a, hist_b]

    with tile.TileContext(nc) as tc:
        with tc.tile_pool(name="pool", bufs=1) as pool:
            # -------- loads --------
            tok0 = pool.tile([P, 1], mybir.dt.int32)
            nc.sync.dma_start(tok0[:], tokens[:, 0:1])
            ib = pool.tile([P, L // 16], mybir.dt.int16)
            nc.sync.dma_start(ib[:], idx_blk[:])
            ic = pool.tile([P, NB // 16], mybir.dt.int16)
            nc.sync.dma_start(ic[:], idx_cat[:])
            isc = pool.tile([P, sc_cols16], mybir.dt.int16)
            nc.sync.dma_start(isc[:], idx_sc[:])
            rl = pool.tile([P, n_cols], mybir.dt.int16)
            nc.sync.dma_start(rl[:], rlow[:])

            # -------- cat_ids output --------
            nc.sync.dma_start(cids[:], tok0[:])

            # -------- one-hot(token % 128) per scatter cell --------
            iota_t = pool.tile([P, D], mybir.dt.int16)
            nc.gpsimd.iota(iota_t[:], pattern=[[1, D]], channel_multiplier=0)
            oh = pool.tile([P, n_cols, D], mybir.dt.float32)
            nc.vector.tensor_tensor(
                out=oh[:],
                in0=iota_t[:].rearrange("p (o d) -> p o d", o=1).to_broadcast([P, n_cols, D]),
                in1=rl[:].to_broadcast([P, n_cols, D]),
                op=AluOpType.is_equal,
            )
            ohf = oh[:].rearrange("p c d -> p (c d)")
            # band 1 -> band 0 (both halves at once)
            merge_insts = [nc.vector.tensor_add(
                ohf[:, 0:BK * D], ohf[:, 0:BK * D], ohf[:, BK * D:2 * BK * D]
            )]
            # overflow levels 1..3 -> level 0
            lv0 = 2 * BK * D
            lw = 2 * ovc * D          # elements per overflow level (both halves)
            for j in range(1, NLEV):
                merge_insts.append(nc.vector.tensor_add(
                    ohf[:, lv0:lv0 + lw],
                    ohf[:, lv0:lv0 + lw],
                    ohf[:, lv0 + j * lw:lv0 + (j + 1) * lw],
                ))

            # -------- optional explicit hist zero-fill --------
            if ZERO_FILL:
                zt = pool.tile([P, ZCHUNK], mybir.dt.float32)
                nc.vector.memset(zt[:], 0.0)
                for h_ in halves:
                    for q in range(V // ZCHUNK):
                        nc.sync.dma_start(
                            h_[:, q * ZCHUNK:(q + 1) * ZCHUNK], zt[0:HB, :]
                        )

            # -------- combined gather (element order i = k*128 + n) --------
            comb = pool.tile([P, BK, 2 * D], mybir.dt.float32)
            nc.gpsimd.dma_gather(comb[:], wcomb[:], ib[:], L, L, 2 * D, single_packet=False)
            catt = pool.tile([P, 1, D], mybir.dt.float32)
            nc.gpsimd.dma_gather(catt[:], wcat[:], ic[:], NB, NB, D)

            # -------- new_seq token part (rows 256..8447) --------
            seq_tok = seq[2 * NB:NS, :].rearrange("(n k) d -> n k d", k=BK)
            nc.sync.dma_start(seq_tok, comb[:, :, 0:D])

            # -------- num_emb = per-block sum of num rows --------
            numc = pool.tile([P, (BK // 2) * D], mybir.dt.float32)
            nv = numc[:].rearrange("p (k d) -> p k d", d=D)
            num0 = nc.vector.tensor_add(
                nv[:, 0:BK // 2, :], comb[:, 0:BK // 2, D:2 * D],
                comb[:, BK // 2:BK, D:2 * D],
            )
            # Ordering-only edge: the num adds block on the gather DMA; without
            # this the scheduler interleaves them BEFORE the one-hot merges in
            # the DVE stream, stalling the scatter chain ~40us behind the
            # gather completion.
            from concourse.tile import add_dep_helper
            for m in merge_insts:
                add_dep_helper(num0.ins, m.ins, sync=False,
                               reason="run one-hot merges before gather-blocked num adds")
            h = BK // 4
            while h >= 1:
                nc.vector.tensor_add(
                    numc[:, 0:h * D], numc[:, 0:h * D], numc[:, h * D:2 * h * D]
                )
                h //= 2
            nc.sync.dma_start(seq[NB:2 * NB, :], numc[:, 0:D])

            # -------- cat_emb rows 0..127 --------
            nc.sync.dma_start(seq[0:NB, :], catt[:, 0, :])

            # -------- hist scatter-add (two independent half-chains) --------
            rows = [h_[:].rearrange("n (s d) -> (n s) d", d=D) for h_ in halves]
            icol = 0
            for name, cap, half in calls:
                if name.startswith("s0"):
                    c0 = half * (HB // 2)                     # 0 or 32
                elif name.startswith("ov"):
                    c0 = 2 * BK + half * ovc                  # level-0 half slice
                else:
                    e = int(name[2:])
                    c0 = extra_col0 + 2 * e + half
                nc.gpsimd.dma_scatter_add(
                    rows[half], oh[:, c0:c0 + cap // P, :],
                    isc[:, icol:icol + cap // 16], cap, cap, D,
                    single_packet=cap <= 1024,
                )
                icol += cap // 16

    nc.compile()
    return nc


def _wrap16(vals):
    """int16 wrapped index layout: idx i at [i%16, i//16], tiled to 128 rows."""
    n = vals.shape[0]
    w = vals.reshape(n // 16, 16).T.astype(np.int16)
    return np.tile(w, (8, 1))


def _scatter_layout(tb, extra_groups):
    """Band/overflow placement for one batch row. Returns (rlow, idx_sc) or
    None if some key is deeper than the configured groups."""
    n_cols, extra_col0 = _cols(extra_groups)
    calls = _sc_caps(extra_groups)
    ovc = OVCAP // P

    rlow = np.full((P, n_cols), -1, np.int16)
    sc = {}

    for half in (0, 1):
        hb0 = half * HB
        th = tb[hb0:hb0 + HB].reshape(-1)                    # 4096 tokens
        n_loc = np.arange(HB * BK, dtype=np.int64) >> 6      # local block
        key = n_loc * SLOTS + (th >> 7)                      # half-local row
        low = (th & 127).astype(np.int16)

        order = np.argsort(key, kind="stable")
        sk = key[order]
        new_run = np.ones(HB * BK, bool)
        new_run[1:] = sk[1:] != sk[:-1]
        run_id = np.cumsum(new_run) - 1
        run_start = np.flatnonzero(new_run)
        occ = np.arange(HB * BK) - run_start[run_id]
        if (occ >= 2 + NLEV + extra_groups).any():
            return None

        c_b0 = half * (HB // 2)                              # 0 / 32
        # occurrence 0 -> band 0; 1 -> band 1
        m0 = run_id[occ == 0]
        rlow[m0 % P, c_b0 + m0 // P] = low[order[occ == 0]]
        m1 = run_id[occ == 1]
        rlow[m1 % P, BK + c_b0 + m1 // P] = low[order[occ == 1]]
        slots0 = np.zeros(L // 2, np.int64)
        slots0[m0] = sk[occ == 0]
        sc["s0a" if half == 0 else "s0b"] = slots0

        # overflow: occ 2..5 -> positional levels; m2 assigned at occ 2
        sel2 = np.flatnonzero(occ == 2)
        if sel2.size > OVCAP:
            return None
        m2_of = np.full(HB * BK, -1, np.int64)               # by sorted pos
        m2_of[sel2] = np.arange(sel2.size)
        ov_slots = np.zeros(OVCAP, np.int64)
        ov_slots[np.arange(sel2.size)] = sk[sel2]
        c_ov0 = 2 * BK + half * ovc
        for j in range(NLEV):
            q = np.flatnonzero(occ == 2 + j)
            if q.size == 0:
                continue
            m2 = m2_of[q - j]                                # occ-2 sibling
            rlow[m2 % P, c_ov0 + j * 2 * ovc + m2 // P] = low[order[q]]
        sc["ova" if half == 0 else "ovb"] = ov_slots

        # extras: occ >= 2+NLEV, one chained cap-128 group per depth level
        for e in range(extra_groups):
            q = np.flatnonzero(occ == 2 + NLEV + e)
            if q.size > P:
                return None
            xs = np.zeros(P, np.int64)
            xs[np.arange(q.size)] = sk[q]
            mg = np.arange(q.size)
            rlow[mg % P, extra_col0 + 2 * e + half] = low[order[q]]
            sc[f"xa{e}" if half == 0 else f"xb{e}"] = xs

    for name, cap, _ in calls:
        if name not in sc:
            sc[name] = np.zeros(cap, np.int64)
    idx_sc = np.concatenate([_wrap16(sc[name]) for name, _, _ in calls], axis=1)
    return rlow, idx_sc


def _make_in_maps(tokens, cat_W, num_W, token_W, extra_groups):
    t32 = np.ascontiguousarray(np.asarray(tokens, dtype=np.int32))
    wcat = np.ascontiguousarray(np.asarray(cat_W, dtype=np.float32))
    wcomb = np.ascontiguousarray(
        np.concatenate(
            [np.asarray(token_W, dtype=np.float32), np.asarray(num_W, dtype=np.float32)],
            axis=1,
        )
    )
    in_maps = []
    for b in range(N_CORES):
        tb = t32[b].reshape(NB, BK)
        lay = _scatter_layout(tb, extra_groups)
        if lay is None:
            return None
        rlow, idx_sc = lay
        in_maps.append({
            "tokens": tb,
            "wcat": wcat,
            "wcomb": wcomb,
            "idx_blk": _wrap16(tb.T.reshape(-1)),
            "idx_cat": _wrap16(tb[:, 0]),
            "idx_sc": idx_sc,
            "rlow": rlow,
        })
    return in_maps


def kernel(tokens, cat_W, num_W, token_W):
    from concourse.bass_utils import run_bass_kernel_spmd

    tokens = np.asarray(tokens)
    tok_dtype = tokens.dtype

    extra = _CACHE.get("extra_groups", 0)
    while True:
        in_maps = _make_in_maps(tokens, cat_W, num_W, token_W, extra)
        if in_maps is not None:
            break
        extra += 2          # pathological duplicate depth: widen chains
    if _CACHE.get("extra_groups") != extra or "nc" not in _CACHE:
        _CACHE["nc"] = build_nc(extra)
        _CACHE["extra_groups"] = extra
    nc = _CACHE["nc"]

    out = run_bass_kernel_spmd(nc, in_maps, core_ids=list(range(N_CORES)))
    _CACHE["last_result"] = out
    results = out.results

    new_seq = np.stack([r["seq"] for r in results])
    cat_ids = np.stack([r["cids"].reshape(NB) for r in results])
    hist = np.stack(
        [np.concatenate([r["hist_a"], r["hist_b"]], axis=0) for r in results]
    )
    return new_seq.astype(np.float32), cat_ids.astype(tok_dtype), hist.astype(np.float32)
